# revision 12
# baseline (speedup 1.0000x reference)
"""InterleavedHeadAttention Trainium2 kernel.

Sharding (8 cores): core c handles batch b = c//4 and 4 output heads
[4*(c%4), 4*(c%4)+4).  The alpha head-mixing einsum is folded into the
QKV projection weights on the host, so each core's projections only
produce its own heads' (h, p, d) slices.  The pseudo-head merge uses
(p, n) flat ordering internally (attention is permutation invariant;
the token-causal mask depends only on n).

Design notes (v3, fp8):
- All GEMMs except AV run as fp8e4m3 DoubleRow matmuls (2 MACs/PE
  cell/cycle): contraction packs two phases per pass via 3D APs
  [K, 2, M] x [K, 2, N].  Weights are host-prescaled by 256 and q/k by
  a further 32 on store; descales fold into existing copy scale
  factors and the exp() activation scale, so they cost nothing.
- Attention inner loop is software-pipelined (depth 2): PE emits scores
  for steps t+1..t+2 before the AV matmuls of step t, hiding the
  PE->Act(exp)->DVE(tri mask)->PE(AV) chain latency.  Projections for
  head h+1 are interleaved into head h's attention steps as PE filler.
- Scores for both pseudo-head queries (pq) land in one [128,1024] PSUM
  pair tile so a single Act exp instruction covers both.
- Q bias fuses into the PSUM->SBUF copy on Act (activation bias+scale),
  K bias on DVE (tensor_scalar); V bias folds into bo on the host
  (attention weights sum to 1 so +bv commutes with attention).
- Softmax 1/den: DVE reciprocal, broadcast across the 64 d-partitions
  by a tiny PE matmul with a constant-32 stationary (which also applies
  the fp8 store scale for ot2), Pool copies PSUM->SBUF.
- Partial output is fp16 (halved DMA); host sums partials in f32.
"""
import numpy as np
import ml_dtypes

import concourse.bacc as bacc
import concourse.bass as bass
import concourse.tile as tile
import concourse.mybir as mybir
from concourse.bass_utils import run_bass_kernel_spmd

B, S, HID, H, P = 2, 1024, 1024, 16, 2
D = HID // H          # 64
HL = 4                # heads per core
G = HL * P            # (h,p) groups per core = 8
HPD = HL * P * D      # 512 projection rows per core
BF = mybir.dt.bfloat16
F32 = mybir.dt.float32
F16 = mybir.dt.float16
FP8 = mybir.dt.float8e4
NCORES = 8
KT2 = HID // 256      # 4 DoubleRow contraction passes over hidden
NT = S // 512         # 2 n windows

SQ = 32.0             # q/k/ot2 fp8 store scale
SW = 256.0            # weight fp8 store scale
EXP_SCALE = 0.125 / (SQ * SQ)

_compiled = None


def _build(reps=1, debug=False):
    nc = bacc.Bacc()
    x8 = nc.dram_tensor("x8", (128, KT2 * 2 * S), FP8, kind="ExternalInput")
    wq8 = nc.dram_tensor("wq8", (128, KT2 * 2 * HPD), FP8, kind="ExternalInput")
    wk8 = nc.dram_tensor("wk8", (128, KT2 * 2 * HPD), FP8, kind="ExternalInput")
    wv8 = nc.dram_tensor("wv8", (128, KT2 * 2 * HPD), FP8, kind="ExternalInput")
    bqT = nc.dram_tensor("bqT", (128, HL), F32, kind="ExternalInput")
    bkT = nc.dram_tensor("bkT", (128, HL), F32, kind="ExternalInput")
    wo8 = nc.dram_tensor("wo8", (64, HL * 2 * HID), FP8, kind="ExternalInput")
    tri = nc.dram_tensor("tri", (128, 128), BF, kind="ExternalInput")
    out = nc.dram_tensor("o", (S, HID), F16, kind="ExternalOutput")
    if debug:
        dbg_qt = nc.dram_tensor("dbg_qt", (64, 2 * S), FP8, kind="ExternalOutput")
        dbg_kt = nc.dram_tensor("dbg_kt", (64, 2 * S), FP8, kind="ExternalOutput")
        dbg_va = nc.dram_tensor("dbg_va", (128, G * 65), BF, kind="ExternalOutput")
        dbg_ot = nc.dram_tensor("dbg_ot", (HL * 64, 2 * S), FP8, kind="ExternalOutput")

    with tile.TileContext(nc) as tc:
        with tc.tile_pool(name="persist", bufs=1) as pp, \
             tc.tile_pool(name="ppl", bufs=4) as ppl, \
             tc.tile_pool(name="sml", bufs=4) as sml, \
             tc.tile_pool(name="smb", bufs=2) as smb, \
             tc.tile_pool(name="osb", bufs=3) as osb, \
             tc.tile_pool(name="ps", bufs=3, space=bass.MemorySpace.PSUM) as ps, \
             tc.tile_pool(name="psav", bufs=2, space=bass.MemorySpace.PSUM) as psav:
          for _rep in range(reps):
            # ---- persistent tiles + input DMAs (first-needed first) ----
            wq_sb = pp.tile([128, KT2 * 2 * HPD], FP8, tag="wq", name="wq_sb")
            nc.gpsimd.dma_start(wq_sb[:], wq8[:])
            x_sb = pp.tile([128, KT2 * 2 * S], FP8, tag="x8", name="x_sb")
            nc.gpsimd.dma_start(x_sb[:], x8[:])
            wk_sb = pp.tile([128, KT2 * 2 * HPD], FP8, tag="wk", name="wk_sb")
            nc.gpsimd.dma_start(wk_sb[:], wk8[:])
            wv_sb = pp.tile([128, KT2 * 2 * HPD], FP8, tag="wv", name="wv_sb")
            nc.gpsimd.dma_start(wv_sb[:], wv8[:])
            w_sb = {"q": wq_sb, "k": wk_sb, "v": wv_sb}
            bqT_sb = pp.tile([128, HL], F32, tag="bqT", name="bqT_sb")
            nc.gpsimd.dma_start(bqT_sb[:], bqT[:])
            bkT_sb = pp.tile([128, HL], F32, tag="bkT", name="bkT_sb")
            nc.gpsimd.dma_start(bkT_sb[:], bkT[:])
            tri_sb = pp.tile([128, 128], BF, tag="tri", name="tri_sb")
            nc.gpsimd.dma_start(tri_sb[:], tri[:])
            c32 = pp.tile([1, 64], BF, tag="c32", name="c32")
            nc.gpsimd.memset(c32[:], SQ)
            woe_sb = pp.tile([64, HL * 2 * HID], FP8, tag="woe", name="woe_sb")
            nc.gpsimd.dma_start(woe_sb[:], wo8[:])

            x4 = x_sb.rearrange("p (k f n) -> p k f n", k=KT2, f=2)
            w4 = {nm: w_sb[nm].rearrange("p (k f c) -> p k f c", k=KT2, f=2)
                  for nm in ("q", "k", "v")}
            woe4 = woe_sb.rearrange("p (h f j) -> p h f j", h=HL, f=2)

            # ---- Q/K projections -> qt8/kt8 [64=(pq,d%32), (f=d//32, S)] fp8
            qt_sb = [pp.tile([64, 2 * S], FP8, tag=f"qt{h}", name=f"qt{h}") for h in range(HL)]
            kt_sb = [pp.tile([64, 2 * S], FP8, tag=f"kt{h}", name=f"kt{h}") for h in range(HL)]
            # kt2 = kt with pk halves swapped (walrus needs matching fmap /
            # weight start partitions for the pq != pk score matmuls)
            kt2_sb = [pp.tile([64, 2 * S], FP8, tag=f"kt2{h}", name=f"kt2{h}") for h in range(HL)]

            def emit_proj(nm, mt):
                acc = ps.tile([128, 1024], F32, tag="big", name="acc")
                for nt in range(NT):
                    for k in range(KT2):
                        nc.tensor.matmul(
                            acc[:, nt * 512:(nt + 1) * 512],
                            w4[nm][:, k, :, mt * 128:(mt + 1) * 128],
                            x4[:, k, :, nt * 512:(nt + 1) * 512],
                            start=(k == 0), stop=(k == KT2 - 1),
                            perf_mode=mybir.MatmulPerfMode.DoubleRow)
                # acc rows (pq,d); store 32*(acc/256) + 32*bias into fp8
                # at [pq*32 + d%32, (d//32)*S + n]
                dst = (qt_sb if nm == "q" else kt_sb)[mt]
                bias = (bqT_sb if nm == "q" else bkT_sb)
                for pq in range(2):
                    for f in range(2):
                        src = acc[pq * 64 + f * 32:pq * 64 + f * 32 + 32, :]
                        dsl = dst[pq * 32:pq * 32 + 32, f * S:(f + 1) * S]
                        bsl = bias[pq * 64 + f * 32:pq * 64 + f * 32 + 32, mt:mt + 1]
                        if nm == "q":
                            nc.scalar.activation(
                                dsl, src, mybir.ActivationFunctionType.Identity,
                                scale=SQ / SW, bias=bsl)
                        else:
                            nc.vector.tensor_scalar(
                                dsl, src, SQ / SW, bsl,
                                mybir.AluOpType.mult, mybir.AluOpType.add)
                if nm == "k":
                    nc.gpsimd.tensor_copy(kt2_sb[mt][0:32, :], kt_sb[mt][32:64, :])
                    nc.gpsimd.tensor_copy(kt2_sb[mt][32:64, :], kt_sb[mt][0:32, :])

            emit_proj("q", 0)
            emit_proj("k", 0)

            # ---- V projection -> vaug [128 n, G*65] bf16 (65th col = ones)
            vaug = [pp.tile([128, G * 65], BF, tag=f"va{j}", name=f"va{j}")
                    for j in range(S // 128)]
            for jp in range(S // 256):
                acc = ps.tile([128, 1024], F32, tag="big", name="acc")
                for half in range(2):
                    jt = jp * 2 + half
                    for k in range(KT2):
                        nc.tensor.matmul(
                            acc[:, half * 512:(half + 1) * 512],
                            x4[:, k, :, jt * 128:(jt + 1) * 128],
                            w4["v"][:, k, :, :],
                            start=(k == 0), stop=(k == KT2 - 1),
                            perf_mode=mybir.MatmulPerfMode.DoubleRow)
                for half in range(2):
                    jt = jp * 2 + half
                    v3 = vaug[jt].rearrange("p (g e) -> p g e", e=65)
                    nc.gpsimd.memset(v3[:, :, 64:65], 1.0)
                    nc.vector.tensor_scalar_mul(
                        v3[:, :, 0:64],
                        acc[:, half * 512:(half + 1) * 512].rearrange(
                            "p (g e) -> p g e", e=64),
                        1.0 / SW)

            # ---- attention: software-pipelined over (h, In, Jn, pk) ----
            # ot2 stored fp8, scaled by 32: [64 d, (f=pq, S)]
            ot2 = [pp.tile([64, 2 * S], FP8, tag=f"ot2{h}", name=f"ot2{h}") for h in range(HL)]
            steps = []
            for h in range(HL):
                for In in range(NT):
                    JN = 4 * In + 4
                    for Jn in range(JN):
                        for pk in range(2):
                            steps.append((h, In, Jn, pk,
                                          Jn == 0 and pk == 0,
                                          Jn == JN - 1 and pk == 1))
            sps, pts, avps = {}, {}, {}
            qt3 = [qt_sb[h].rearrange("p (f n) -> p f n", f=2) for h in range(HL)]
            kt3 = [kt_sb[h].rearrange("p (f n) -> p f n", f=2) for h in range(HL)]
            kt23 = [kt2_sb[h].rearrange("p (f n) -> p f n", f=2) for h in range(HL)]

            def tri_bcast():
                t0 = tri_sb[:]
                return bass.AP(tri_sb.tensor, t0.offset, [t0.ap[0], [0, 2], t0.ap[1]])

            def emit_scores(t):
                h, In, Jn, pk, st, fin = steps[t]
                FF = 128 * (Jn - 4 * In)
                c0 = FF if FF >= 0 else 0
                sp2 = ps.tile([128, 1024], F32, tag="big", name="sp")
                jsl = slice(Jn * 128, (Jn + 1) * 128)
                isl = slice(In * 512 + c0, (In + 1) * 512)
                for pq in range(2):
                    ksrc = kt3[h] if pk == pq else kt23[h]
                    nc.tensor.matmul(
                        sp2[:, pq * 512 + c0:(pq + 1) * 512],
                        ksrc[pq * 32:(pq + 1) * 32, :, jsl],
                        qt3[h][pq * 32:(pq + 1) * 32, :, isl],
                        start=True, stop=True, tile_position=(pq * 32, 0),
                        perf_mode=mybir.MatmulPerfMode.DoubleRow)
                sps[t] = sp2

            def emit_exp(t):
                h, In, Jn, pk, st, fin = steps[t]
                FF = 128 * (Jn - 4 * In)
                c0 = FF if FF >= 0 else 0
                pt = ppl.tile([128, 1024], BF, tag="pt", name="pt")
                sp3 = sps[t].rearrange("p (q n) -> p q n", q=2)
                pt3 = pt.rearrange("p (q n) -> p q n", q=2)
                nc.scalar.activation(
                    pt3[:, :, c0:512], sp3[:, :, c0:512],
                    mybir.ActivationFunctionType.Exp, scale=EXP_SCALE)
                if FF >= 0:
                    nc.vector.tensor_mul(
                        pt3[:, :, c0:c0 + 128], pt3[:, :, c0:c0 + 128], tri_bcast())
                pts[t] = pt

            def emit_av(t):
                h, In, Jn, pk, st, fin = steps[t]
                FF = 128 * (Jn - 4 * In)
                c0 = FF if FF >= 0 else 0
                if st:
                    avps[(h, In)] = [
                        psav.tile([65, 512], F32, tag="av", name="av")
                        for _ in range(2)]
                avp = avps[(h, In)]
                g = h * 2 + pk
                pt3 = pts[t].rearrange("p (q n) -> p q n", q=2)
                for pq in range(2):
                    nc.tensor.matmul(
                        avp[pq][:, c0:512],
                        vaug[Jn][:, g * 65:g * 65 + 65],
                        pt3[:, pq, c0:512],
                        start=st, stop=fin)
                if fin:
                    emit_norm(h, In)

            def emit_norm(h, In):
                avp = avps[(h, In)]
                # bcs = 32/den broadcast across 64 partitions (PE matmul with
                # constant-32 stationary also applies the fp8 store scale)
                bcp = ps.tile([128, 1024], F32, tag="big", name="bcp")
                for pq in range(2):
                    rc = sml.tile([1, 512], BF, tag="rc", name="rc")
                    with nc.allow_low_precision(reason="softmax recip bf16"):
                        nc.vector.reciprocal(rc[:], avp[pq][64:65, :])
                    nc.tensor.matmul(
                        bcp[pq * 64:(pq + 1) * 64, 0:512], c32[:], rc[:],
                        start=True, stop=True, tile_position=(0, pq * 64))
                bcs = smb.tile([128, 512], BF, tag="bcs", name="bcs")
                nc.vector.tensor_copy(bcs[:], bcp[:, 0:512])
                for pq in range(2):
                    nc.vector.tensor_mul(
                        ot2[h][0:64, pq * S + In * 512:pq * S + (In + 1) * 512],
                        avp[pq][0:64, :], bcs[pq * 64:(pq + 1) * 64, :])

            DP = 2
            T = len(steps)
            STEPS_PER_HEAD = sum(2 * (4 * In + 4) for In in range(NT))  # 24
            for t in range(T):
                h = steps[t][0]
                local = t - h * STEPS_PER_HEAD
                if h < HL - 1:
                    if local == 6:
                        emit_proj("q", h + 1)
                    elif local == 14:
                        emit_proj("k", h + 1)
                emit_scores(t)
                emit_exp(t)
                if t >= DP:
                    emit_av(t - DP)
            for t in range(T - DP, T):
                emit_av(t)

            if debug:
                nc.gpsimd.dma_start(dbg_qt[:], qt_sb[0][:])
                nc.gpsimd.dma_start(dbg_kt[:], kt_sb[0][:])
                nc.gpsimd.dma_start(dbg_va[:], vaug[0][:])
                for hh in range(HL):
                    nc.gpsimd.dma_start(dbg_ot[hh * 64:(hh + 1) * 64, :], ot2[hh][:])

            # ---- output projection (fp8 DoubleRow over (pq,d)) ----
            ot3 = [ot2[h].rearrange("p (f n) -> p f n", f=2) for h in range(HL)]
            for mt in range(S // 128):
                op = ps.tile([128, 1024], F32, tag="big", name="op")
                for jt in range(HID // 512):
                    for h in range(HL):
                        nc.tensor.matmul(
                            op[:, jt * 512:(jt + 1) * 512],
                            ot3[h][:, :, mt * 128:(mt + 1) * 128],
                            woe4[:, h, :, jt * 512:(jt + 1) * 512],
                            start=(h == 0), stop=(h == HL - 1),
                            perf_mode=mybir.MatmulPerfMode.DoubleRow)
                ob = osb.tile([128, 1024], F16, tag="ob", name="ob")
                nc.scalar.activation(
                    ob[:], op[:], mybir.ActivationFunctionType.Identity,
                    scale=1.0 / (SQ * SW))
                nc.gpsimd.dma_start(out[mt * 128:(mt + 1) * 128, :], ob[:])
    nc.compile()
    return nc


def _prep(inputs):
    f8 = ml_dtypes.float8_e4m3
    hs = np.asarray(inputs["hidden_states"], np.float32)
    maps = []
    tri = np.triu(np.ones((128, 128), np.float32)).astype(ml_dtypes.bfloat16)
    eff = {}
    for nm in ("q", "k", "v"):
        W = np.asarray(inputs[f"W{nm}"], np.float32)
        bb = np.asarray(inputs[f"b{nm}"], np.float32)
        al = np.asarray(inputs[f"alpha_{nm}"], np.float32)
        We = np.einsum("mhp,mdc->hpdc", al, W.reshape(H, D, HID))
        be = np.einsum("mhp,md->hpd", al, bb.reshape(H, D))
        eff[nm] = (We, be)
    Wo = np.asarray(inputs["Wo"], np.float32)
    col = np.asarray(inputs["collapse"], np.float32)
    Woe = np.einsum("hp,jhd->hpdj", col, Wo.reshape(HID, H, D))  # (H,P,D,HID)

    def pack_w(WT):  # (HID, c) -> [128, (k,2,c)] prescaled by SW
        c = WT.shape[1]
        return np.ascontiguousarray(
            (SW * WT).reshape(KT2, 2, 128, c).transpose(2, 0, 1, 3)
        ).reshape(128, KT2 * 2 * c).astype(f8)

    for cidx in range(NCORES):
        b, g = cidx // 4, cidx % 4
        hs_sl = slice(g * HL, (g + 1) * HL)
        xT = np.ascontiguousarray(hs[b].T)              # (HID, S)
        m = {"x8": np.ascontiguousarray(
                xT.reshape(KT2, 2, 128, S).transpose(2, 0, 1, 3)
             ).reshape(128, KT2 * 2 * S).astype(f8),
             "tri": tri}
        for nm in ("q", "k", "v"):
            We, be = eff[nm]
            Wslice = We[hs_sl].reshape(HPD, HID)        # (hpd, HID)
            m[f"w{nm}8"] = pack_w(np.ascontiguousarray(Wslice.T))
            if nm != "v":
                m[f"b{nm}T"] = np.ascontiguousarray(
                    SQ * be[hs_sl].reshape(HL, 128).T).astype(np.float32)
        woc = Woe[hs_sl].reshape(HL, 2 * D, HID)        # (HL, 128, HID)
        m["wo8"] = np.ascontiguousarray(
            (SW * woc).reshape(HL, 2, 64, HID).transpose(2, 0, 1, 3)
        ).reshape(64, HL * 2 * HID).astype(f8)
        maps.append(m)
    return maps


def kernel(**inputs):
    global _compiled
    if _compiled is None:
        _compiled = _build()
    maps = _prep(inputs)
    res = run_bass_kernel_spmd(_compiled, maps, core_ids=list(range(NCORES)))
    bo = np.asarray(inputs["bo"], np.float32)
    # V bias folded out of the kernel: attention weights sum to 1, so the
    # +bv term passes through attention unchanged and lands in bo here.
    bv = np.asarray(inputs["bv"], np.float32)
    al = np.asarray(inputs["alpha_v"], np.float32)
    bve = np.einsum("mhp,md->hpd", al, bv.reshape(H, D))
    Wo = np.asarray(inputs["Wo"], np.float32)
    col = np.asarray(inputs["collapse"], np.float32)
    Woe = np.einsum("hp,jhd->hpdj", col, Wo.reshape(HID, H, D))
    bo_eff = bo + np.einsum("hpd,hpdj->j", bve, Woe)
    out = np.zeros((B, S, HID), np.float32)
    for c in range(NCORES):
        out[c // 4] += res.results[c]["o"].astype(np.float32)
    out += bo_eff
    return out


# revision 13
# speedup vs baseline: 1.0335x; 1.0335x over previous
"""InterleavedHeadAttention Trainium2 kernel.

Sharding (8 cores): core c handles batch b = c//4 and 4 output heads
[4*(c%4), 4*(c%4)+4).  The alpha head-mixing einsum is folded into the
QKV projection weights on the host, so each core's projections only
produce its own heads' (h, p, d) slices.  The pseudo-head merge uses
(p, n) flat ordering internally (attention is permutation invariant;
the token-causal mask depends only on n).

Design notes (v3, fp8):
- All GEMMs except AV run as fp8e4m3 DoubleRow matmuls (2 MACs/PE
  cell/cycle): contraction packs two phases per pass via 3D APs
  [K, 2, M] x [K, 2, N].  Weights are host-prescaled by 256 and q/k by
  a further 32 on store; descales fold into existing copy scale
  factors and the exp() activation scale, so they cost nothing.
- Attention inner loop is software-pipelined (depth 2): PE emits scores
  for steps t+1..t+2 before the AV matmuls of step t, hiding the
  PE->Act(exp)->DVE(tri mask)->PE(AV) chain latency.  Projections for
  head h+1 are interleaved into head h's attention steps as PE filler.
- Scores for both pseudo-head queries (pq) land in one [128,1024] PSUM
  pair tile so a single Act exp instruction covers both.
- Q bias fuses into the PSUM->SBUF copy on Act (activation bias+scale),
  K bias on DVE (tensor_scalar); V bias folds into bo on the host
  (attention weights sum to 1 so +bv commutes with attention).
- Softmax 1/den: DVE reciprocal, broadcast across the 64 d-partitions
  by a tiny PE matmul with a constant-32 stationary (which also applies
  the fp8 store scale for ot2), Pool copies PSUM->SBUF.
- Partial output is fp16 (halved DMA); host sums partials in f32.
"""
import numpy as np
import ml_dtypes

import concourse.bacc as bacc
import concourse.bass as bass
import concourse.tile as tile
import concourse.mybir as mybir
from concourse.bass_utils import run_bass_kernel_spmd

B, S, HID, H, P = 2, 1024, 1024, 16, 2
D = HID // H          # 64
HL = 4                # heads per core
G = HL * P            # (h,p) groups per core = 8
HPD = HL * P * D      # 512 projection rows per core
BF = mybir.dt.bfloat16
F32 = mybir.dt.float32
F16 = mybir.dt.float16
FP8 = mybir.dt.float8e4
NCORES = 8
KT2 = HID // 256      # 4 DoubleRow contraction passes over hidden
NT = S // 512         # 2 n windows

SQ = 32.0             # q/k/ot2 fp8 store scale
SW = 256.0            # weight fp8 store scale
EXP_SCALE = 0.125

_compiled = None


def _build(reps=1, debug=False):
    nc = bacc.Bacc()
    x8 = nc.dram_tensor("x8", (128, KT2 * 2 * S), FP8, kind="ExternalInput")
    wq8 = nc.dram_tensor("wq8", (128, KT2 * 2 * HPD), FP8, kind="ExternalInput")
    wk8 = nc.dram_tensor("wk8", (128, KT2 * 2 * HPD), FP8, kind="ExternalInput")
    wv8 = nc.dram_tensor("wv8", (128, KT2 * 2 * HPD), FP8, kind="ExternalInput")
    bqT = nc.dram_tensor("bqT", (128, HL), F32, kind="ExternalInput")
    bkT = nc.dram_tensor("bkT", (128, HL), F32, kind="ExternalInput")
    wo8 = nc.dram_tensor("wo8", (64, HL * 2 * HID), FP8, kind="ExternalInput")
    tri = nc.dram_tensor("tri", (128, 128), BF, kind="ExternalInput")
    out = nc.dram_tensor("o", (S, HID), F16, kind="ExternalOutput")
    if debug:
        dbg_qt = nc.dram_tensor("dbg_qt", (128, S), BF, kind="ExternalOutput")
        dbg_kt = nc.dram_tensor("dbg_kt", (128, S), BF, kind="ExternalOutput")
        dbg_va = nc.dram_tensor("dbg_va", (128, G * 65), BF, kind="ExternalOutput")
        dbg_ot = nc.dram_tensor("dbg_ot", (HL * 64, 2 * S), FP8, kind="ExternalOutput")

    with tile.TileContext(nc) as tc:
        with tc.tile_pool(name="persist", bufs=1) as pp, \
             tc.tile_pool(name="ppl", bufs=4) as ppl, \
             tc.tile_pool(name="sml", bufs=4) as sml, \
             tc.tile_pool(name="smb", bufs=2) as smb, \
             tc.tile_pool(name="osb", bufs=3) as osb, \
             tc.tile_pool(name="ps", bufs=3, space=bass.MemorySpace.PSUM) as ps, \
             tc.tile_pool(name="psav", bufs=2, space=bass.MemorySpace.PSUM) as psav:
          for _rep in range(reps):
            # ---- persistent tiles + input DMAs (first-needed first) ----
            wq_sb = pp.tile([128, KT2 * 2 * HPD], FP8, tag="wq", name="wq_sb")
            nc.gpsimd.dma_start(wq_sb[:], wq8[:])
            x_sb = pp.tile([128, KT2 * 2 * S], FP8, tag="x8", name="x_sb")
            nc.gpsimd.dma_start(x_sb[:], x8[:])
            wk_sb = pp.tile([128, KT2 * 2 * HPD], FP8, tag="wk", name="wk_sb")
            nc.gpsimd.dma_start(wk_sb[:], wk8[:])
            wv_sb = pp.tile([128, KT2 * 2 * HPD], FP8, tag="wv", name="wv_sb")
            nc.gpsimd.dma_start(wv_sb[:], wv8[:])
            w_sb = {"q": wq_sb, "k": wk_sb, "v": wv_sb}
            bqT_sb = pp.tile([128, HL], F32, tag="bqT", name="bqT_sb")
            nc.gpsimd.dma_start(bqT_sb[:], bqT[:])
            bkT_sb = pp.tile([128, HL], F32, tag="bkT", name="bkT_sb")
            nc.gpsimd.dma_start(bkT_sb[:], bkT[:])
            tri_sb = pp.tile([128, 128], BF, tag="tri", name="tri_sb")
            nc.gpsimd.dma_start(tri_sb[:], tri[:])
            c32 = pp.tile([1, 64], BF, tag="c32", name="c32")
            nc.gpsimd.memset(c32[:], SQ)
            woe_sb = pp.tile([64, HL * 2 * HID], FP8, tag="woe", name="woe_sb")
            nc.gpsimd.dma_start(woe_sb[:], wo8[:])

            x4 = x_sb.rearrange("p (k f n) -> p k f n", k=KT2, f=2)
            w4 = {nm: w_sb[nm].rearrange("p (k f c) -> p k f c", k=KT2, f=2)
                  for nm in ("q", "k", "v")}
            woe4 = woe_sb.rearrange("p (h f j) -> p h f j", h=HL, f=2)

            # ---- Q/K projections -> qt/kt [128=(pq,d), S] bf16 ----
            qt_sb = [pp.tile([128, S], BF, tag=f"qt{h}", name=f"qt{h}") for h in range(HL)]
            kt_sb = [pp.tile([128, S], BF, tag=f"kt{h}", name=f"kt{h}") for h in range(HL)]
            # kt2 = kt with pk halves swapped (walrus needs matching fmap /
            # weight start partitions for the pq != pk score matmuls)
            kt2_sb = [pp.tile([128, S], BF, tag=f"kt2{h}", name=f"kt2{h}") for h in range(HL)]

            def emit_proj(nm, mt):
                acc = ps.tile([128, 1024], F32, tag="big", name="acc")
                for nt in range(NT):
                    for k in range(KT2):
                        nc.tensor.matmul(
                            acc[:, nt * 512:(nt + 1) * 512],
                            w4[nm][:, k, :, mt * 128:(mt + 1) * 128],
                            x4[:, k, :, nt * 512:(nt + 1) * 512],
                            start=(k == 0), stop=(k == KT2 - 1),
                            perf_mode=mybir.MatmulPerfMode.DoubleRow)
                # acc rows (pq,d); store acc/256 + bias into bf16 qt/kt
                if nm == "q":
                    nc.scalar.activation(
                        qt_sb[mt][:], acc[:],
                        mybir.ActivationFunctionType.Identity,
                        scale=1.0 / SW, bias=bqT_sb[:, mt:mt + 1])
                else:
                    nc.vector.tensor_scalar(
                        kt_sb[mt][:], acc[:], 1.0 / SW, bkT_sb[:, mt:mt + 1],
                        mybir.AluOpType.mult, mybir.AluOpType.add)
                    nc.gpsimd.tensor_copy(kt2_sb[mt][0:64, :], kt_sb[mt][64:128, :])
                    nc.gpsimd.tensor_copy(kt2_sb[mt][64:128, :], kt_sb[mt][0:64, :])

            emit_proj("q", 0)
            emit_proj("k", 0)

            # ---- V projection -> vaug [128 n, G*65] bf16 (65th col = ones)
            vaug = [pp.tile([128, G * 65], BF, tag=f"va{j}", name=f"va{j}")
                    for j in range(S // 128)]
            for jp in range(S // 256):
                acc = ps.tile([128, 1024], F32, tag="big", name="acc")
                for half in range(2):
                    jt = jp * 2 + half
                    for k in range(KT2):
                        nc.tensor.matmul(
                            acc[:, half * 512:(half + 1) * 512],
                            x4[:, k, :, jt * 128:(jt + 1) * 128],
                            w4["v"][:, k, :, :],
                            start=(k == 0), stop=(k == KT2 - 1),
                            perf_mode=mybir.MatmulPerfMode.DoubleRow)
                for half in range(2):
                    jt = jp * 2 + half
                    v3 = vaug[jt].rearrange("p (g e) -> p g e", e=65)
                    nc.gpsimd.memset(v3[:, :, 64:65], 1.0)
                    nc.vector.tensor_scalar_mul(
                        v3[:, :, 0:64],
                        acc[:, half * 512:(half + 1) * 512].rearrange(
                            "p (g e) -> p g e", e=64),
                        1.0 / SW)

            # ---- attention: software-pipelined over (h, In, Jn, pk) ----
            # ot2 stored fp8, scaled by 32: [64 d, (f=pq, S)]
            ot2 = [pp.tile([64, 2 * S], FP8, tag=f"ot2{h}", name=f"ot2{h}") for h in range(HL)]
            steps = []
            for h in range(HL):
                for In in range(NT):
                    JN = 4 * In + 4
                    for Jn in range(JN):
                        for pk in range(2):
                            steps.append((h, In, Jn, pk,
                                          Jn == 0 and pk == 0,
                                          Jn == JN - 1 and pk == 1))
            sps, pts, avps = {}, {}, {}

            def tri_bcast():
                t0 = tri_sb[:]
                return bass.AP(tri_sb.tensor, t0.offset, [t0.ap[0], [0, 2], t0.ap[1]])

            def emit_scores(t):
                h, In, Jn, pk, st, fin = steps[t]
                FF = 128 * (Jn - 4 * In)
                c0 = FF if FF >= 0 else 0
                sp2 = ps.tile([128, 1024], F32, tag="big", name="sp")
                jsl = slice(Jn * 128, (Jn + 1) * 128)
                isl = slice(In * 512 + c0, (In + 1) * 512)
                for pq in range(2):
                    ksrc = kt_sb[h] if pk == pq else kt2_sb[h]
                    nc.tensor.matmul(
                        sp2[:, pq * 512 + c0:(pq + 1) * 512],
                        ksrc[pq * 64:(pq + 1) * 64, jsl],
                        qt_sb[h][pq * 64:(pq + 1) * 64, isl],
                        start=True, stop=True, tile_position=(pq * 64, 0))
                sps[t] = sp2

            def emit_exp(t):
                h, In, Jn, pk, st, fin = steps[t]
                FF = 128 * (Jn - 4 * In)
                c0 = FF if FF >= 0 else 0
                pt = ppl.tile([128, 1024], BF, tag="pt", name="pt")
                sp3 = sps[t].rearrange("p (q n) -> p q n", q=2)
                pt3 = pt.rearrange("p (q n) -> p q n", q=2)
                nc.scalar.activation(
                    pt3[:, :, c0:512], sp3[:, :, c0:512],
                    mybir.ActivationFunctionType.Exp, scale=EXP_SCALE)
                if FF >= 0:
                    nc.vector.tensor_mul(
                        pt3[:, :, c0:c0 + 128], pt3[:, :, c0:c0 + 128], tri_bcast())
                pts[t] = pt

            def emit_av(t):
                h, In, Jn, pk, st, fin = steps[t]
                FF = 128 * (Jn - 4 * In)
                c0 = FF if FF >= 0 else 0
                if st:
                    avps[(h, In)] = [
                        psav.tile([65, 512], F32, tag="av", name="av")
                        for _ in range(2)]
                avp = avps[(h, In)]
                g = h * 2 + pk
                pt3 = pts[t].rearrange("p (q n) -> p q n", q=2)
                for pq in range(2):
                    nc.tensor.matmul(
                        avp[pq][:, c0:512],
                        vaug[Jn][:, g * 65:g * 65 + 65],
                        pt3[:, pq, c0:512],
                        start=st, stop=fin)
                if fin:
                    emit_norm(h, In)

            def emit_norm(h, In):
                avp = avps[(h, In)]
                # bcs = 32/den broadcast across 64 partitions (PE matmul with
                # constant-32 stationary also applies the fp8 store scale)
                bcp = ps.tile([128, 1024], F32, tag="big", name="bcp")
                for pq in range(2):
                    rc = sml.tile([1, 512], BF, tag="rc", name="rc")
                    with nc.allow_low_precision(reason="softmax recip bf16"):
                        nc.vector.reciprocal(rc[:], avp[pq][64:65, :])
                    nc.tensor.matmul(
                        bcp[pq * 64:(pq + 1) * 64, 0:512], c32[:], rc[:],
                        start=True, stop=True, tile_position=(0, pq * 64))
                bcs = smb.tile([128, 512], BF, tag="bcs", name="bcs")
                nc.vector.tensor_copy(bcs[:], bcp[:, 0:512])
                for pq in range(2):
                    nc.vector.tensor_mul(
                        ot2[h][0:64, pq * S + In * 512:pq * S + (In + 1) * 512],
                        avp[pq][0:64, :], bcs[pq * 64:(pq + 1) * 64, :])

            DP = 2
            T = len(steps)
            STEPS_PER_HEAD = sum(2 * (4 * In + 4) for In in range(NT))  # 24
            for t in range(T):
                h = steps[t][0]
                local = t - h * STEPS_PER_HEAD
                if h < HL - 1:
                    if local == 6:
                        emit_proj("q", h + 1)
                    elif local == 14:
                        emit_proj("k", h + 1)
                emit_scores(t)
                emit_exp(t)
                if t >= DP:
                    emit_av(t - DP)
            for t in range(T - DP, T):
                emit_av(t)

            if debug:
                nc.gpsimd.dma_start(dbg_qt[:], qt_sb[0][:])
                nc.gpsimd.dma_start(dbg_kt[:], kt_sb[0][:])
                nc.gpsimd.dma_start(dbg_va[:], vaug[0][:])
                for hh in range(HL):
                    nc.gpsimd.dma_start(dbg_ot[hh * 64:(hh + 1) * 64, :], ot2[hh][:])

            # ---- output projection (fp8 DoubleRow over (pq,d)) ----
            ot3 = [ot2[h].rearrange("p (f n) -> p f n", f=2) for h in range(HL)]
            for mt in range(S // 128):
                op = ps.tile([128, 1024], F32, tag="big", name="op")
                for jt in range(HID // 512):
                    for h in range(HL):
                        nc.tensor.matmul(
                            op[:, jt * 512:(jt + 1) * 512],
                            ot3[h][:, :, mt * 128:(mt + 1) * 128],
                            woe4[:, h, :, jt * 512:(jt + 1) * 512],
                            start=(h == 0), stop=(h == HL - 1),
                            perf_mode=mybir.MatmulPerfMode.DoubleRow)
                ob = osb.tile([128, 1024], F16, tag="ob", name="ob")
                nc.scalar.activation(
                    ob[:], op[:], mybir.ActivationFunctionType.Identity,
                    scale=1.0 / (SQ * SW))
                nc.gpsimd.dma_start(out[mt * 128:(mt + 1) * 128, :], ob[:])
    nc.compile()
    return nc


def _prep(inputs):
    f8 = ml_dtypes.float8_e4m3
    hs = np.asarray(inputs["hidden_states"], np.float32)
    maps = []
    tri = np.triu(np.ones((128, 128), np.float32)).astype(ml_dtypes.bfloat16)
    eff = {}
    for nm in ("q", "k", "v"):
        W = np.asarray(inputs[f"W{nm}"], np.float32)
        bb = np.asarray(inputs[f"b{nm}"], np.float32)
        al = np.asarray(inputs[f"alpha_{nm}"], np.float32)
        We = np.einsum("mhp,mdc->hpdc", al, W.reshape(H, D, HID))
        be = np.einsum("mhp,md->hpd", al, bb.reshape(H, D))
        eff[nm] = (We, be)
    Wo = np.asarray(inputs["Wo"], np.float32)
    col = np.asarray(inputs["collapse"], np.float32)
    Woe = np.einsum("hp,jhd->hpdj", col, Wo.reshape(HID, H, D))  # (H,P,D,HID)

    def pack_w(WT):  # (HID, c) -> [128, (k,2,c)] prescaled by SW
        c = WT.shape[1]
        return np.ascontiguousarray(
            (SW * WT).reshape(KT2, 2, 128, c).transpose(2, 0, 1, 3)
        ).reshape(128, KT2 * 2 * c).astype(f8)

    for cidx in range(NCORES):
        b, g = cidx // 4, cidx % 4
        hs_sl = slice(g * HL, (g + 1) * HL)
        xT = np.ascontiguousarray(hs[b].T)              # (HID, S)
        m = {"x8": np.ascontiguousarray(
                xT.reshape(KT2, 2, 128, S).transpose(2, 0, 1, 3)
             ).reshape(128, KT2 * 2 * S).astype(f8),
             "tri": tri}
        for nm in ("q", "k", "v"):
            We, be = eff[nm]
            Wslice = We[hs_sl].reshape(HPD, HID)        # (hpd, HID)
            m[f"w{nm}8"] = pack_w(np.ascontiguousarray(Wslice.T))
            if nm != "v":
                m[f"b{nm}T"] = np.ascontiguousarray(
                    be[hs_sl].reshape(HL, 128).T).astype(np.float32)
        woc = Woe[hs_sl].reshape(HL, 2 * D, HID)        # (HL, 128, HID)
        m["wo8"] = np.ascontiguousarray(
            (SW * woc).reshape(HL, 2, 64, HID).transpose(2, 0, 1, 3)
        ).reshape(64, HL * 2 * HID).astype(f8)
        maps.append(m)
    return maps


def kernel(**inputs):
    global _compiled
    if _compiled is None:
        _compiled = _build()
    maps = _prep(inputs)
    res = run_bass_kernel_spmd(_compiled, maps, core_ids=list(range(NCORES)))
    bo = np.asarray(inputs["bo"], np.float32)
    # V bias folded out of the kernel: attention weights sum to 1, so the
    # +bv term passes through attention unchanged and lands in bo here.
    bv = np.asarray(inputs["bv"], np.float32)
    al = np.asarray(inputs["alpha_v"], np.float32)
    bve = np.einsum("mhp,md->hpd", al, bv.reshape(H, D))
    Wo = np.asarray(inputs["Wo"], np.float32)
    col = np.asarray(inputs["collapse"], np.float32)
    Woe = np.einsum("hp,jhd->hpdj", col, Wo.reshape(HID, H, D))
    bo_eff = bo + np.einsum("hpd,hpdj->j", bve, Woe)
    out = np.zeros((B, S, HID), np.float32)
    for c in range(NCORES):
        out[c // 4] += res.results[c]["o"].astype(np.float32)
    out += bo_eff
    return out
